# revision 42
# baseline (speedup 1.0000x reference)
"""Multi-head causal attention (b=2, s=2048, d=1024, h=16) on 8 TRN2 cores.

Sharding: batch (2) x head-groups (4 heads each) -> 8 cores, Megatron-style.
Each core: QKV col-sliced projections (d -> 256), causal attention for its 4
heads, row-sliced output projection producing a partial [2048, 1024] output.
Host sums the 4 partials per batch and adds the output bias.

v3 design (fused streaming pipeline, col-tiled ctx):
  - One pass over s in 4 chunks of 512: projections for chunk sc are emitted
    interleaved with attention for query block sc-1, so PE, ACT (exp), DVE
    and GPSIMD stay busy end-to-end.
  - Heads processed in PAIRS stacked on partition halves: q/k projections are
    unpadded [d,128] blocks, scores for the two heads of a pair run
    CONCURRENTLY as row-tiled K=64 matmuls (tile_position (0,0)/(64,0)).
  - The two concurrent score matmuls write one [128, 2, 512] PSUM supertile
    (2 banks) so a single ACT exp instruction covers both heads.
  - ctx matmuls for the pair are COLUMN-tiled (M=64 each, tile_position
    (0,0)/(0,64)) into ONE [128, 512] PSUM bank and run concurrently --
    half the PE time of the sequential M=65 variant.
  - Softmax denominator: DVE accumulates pden[keys, 2, q] += pt per unit
    (bf16, 2x/4x DVE modes); at group end one col-tiled pair of
    ones[128,64]^T @ pden matmuls reduces over keys AND broadcasts the
    denominator to all 128 partitions in one shot; recip + one [128,512]
    tensor_mul normalizes both heads at once.
  - Causal mask via gpsimd.affine_select on the exp output, restricted to
    the 128-wide diagonal band (the only region a diagonal unit can mask).
  - PE warmup: a short spin of dummy matmuls at kernel start keeps the HAM
    activity monitor busy through the DMA prologue so real matmuls start at
    2.4 GHz (warm) instead of 1.2 GHz (cold).
  - Dual DMA queues: inputs stream on the sync HWDGE queue, outputs on the
    scalar HWDGE queue, so the 8 MB of output writes never queue ahead of
    input prefetches.
"""

import ml_dtypes
import numpy as np

import concourse.bass as bass
import concourse.tile as tile
from concourse import bacc
from concourse import mybir
from concourse import bass_utils

F32 = mybir.dt.float32
BF16 = mybir.dt.bfloat16
EXP = mybir.ActivationFunctionType.Exp

B, S, D, H = 2, 2048, 1024, 16
HG = 4                  # heads per core
E = 64                  # head dim
DG = HG * E             # 256, d-slice per core
NC = 8                  # cores
IT = 512                # query block (moving dim of attention matmuls)
JT = 128                # key tile
KC = D // 128           # 8 contraction chunks for projections
NSC = S // IT           # 4 s-chunks of 512
SCALE = 1.0 / np.sqrt(E)
SK = 3                  # ctx-matmul skew (att pipeline depth, in units)
NWARM = 26              # HAM warmup dummy matmuls

_CACHE = {}


def _build():
    nc = bacc.Bacc("TRN2", target_bir_lowering=False, debug=False)

    xp = nc.dram_tensor("xp", [128, NSC * KC * IT], BF16, kind="ExternalInput").ap()
    wq = nc.dram_tensor("wq", [128, KC * DG], BF16, kind="ExternalInput").ap()
    wk = nc.dram_tensor("wk", [128, KC * DG], BF16, kind="ExternalInput").ap()
    wv = nc.dram_tensor("wv", [128, KC * DG], BF16, kind="ExternalInput").ap()
    wo = nc.dram_tensor("wo", [128, 2 * D], BF16, kind="ExternalInput").ap()
    wf = nc.dram_tensor("wf", [128, 2 * DG + 128], BF16, kind="ExternalInput").ap()
    out = nc.dram_tensor("out", [S, D], BF16, kind="ExternalOutput").ap()

    with tile.TileContext(nc) as tc:
        from contextlib import ExitStack

        with ExitStack() as ctx:
            pers = ctx.enter_context(tc.tile_pool(name="pers", bufs=1))

            wq_sb = pers.tile([128, KC * DG], BF16, tag="wq")         # 4 KB/p
            wk_sb = pers.tile([128, KC * DG], BF16, tag="wk")
            wv_sb = pers.tile([128, KC * DG], BF16, tag="wv")
            wo_sb = pers.tile([128, 2 * D], BF16, tag="wo")
            qT_sb = pers.tile([128, 2 * S], BF16, tag="qT")           # 8 KB/p
            kT_sb = pers.tile([128, 2 * S], BF16, tag="kT")
            v_sb = pers.tile([128, 16 * HG * E], BF16, tag="v")       # 8 KB/p
            cx_sb = pers.tile([128, 2 * S], BF16, tag="cx")

            v4 = v_sb.rearrange("p (t h x) -> p t h x", t=16, h=HG)
            VW = HG * E  # 256

            xs_pool = ctx.enter_context(tc.tile_pool(name="xsp", bufs=3))
            pt_pool = ctx.enter_context(tc.tile_pool(name="ptp", bufs=9))
            ot_pool = ctx.enter_context(tc.tile_pool(name="otp", bufs=5))
            dn_pool = ctx.enter_context(tc.tile_pool(name="dnp", bufs=2))
            sp_pool = ctx.enter_context(tc.tile_pool(name="spp", bufs=2, space="PSUM"))
            # cps is one bank per pair-group now (col-tiled ctx): 2 banks
            # of pipeline depth. proj groups / out-proj / den-broadcast
            # share a double-buffered ring so the PE never WAR-stalls on the
            # staging copy of the previous group.
            cp_pool = ctx.enter_context(tc.tile_pool(name="cpp", bufs=2, space="PSUM"))
            pj_pool = ctx.enter_context(tc.tile_pool(name="pjp", bufs=2, space="PSUM"))

            zero_reg = nc.gpsimd.to_reg(0.0)

            # ---- HAM warmup: keep the PE busy through the DMA prologue so
            # the clock gate opens (K=8/8) before real matmuls arrive.
            # gpsimd memset: its preamble drains earliest, so the spin can
            # start ~2us sooner than with a DVE memset ----
            dummy_sb = pers.tile([128, IT], BF16, tag="dummy")
            nc.gpsimd.memset(dummy_sb[:], 0.0)
            wps = pj_pool.tile([128, IT], F32, name="warm", tag="pj")
            for _ in range(NWARM):
                nc.tensor.matmul(wps[:], lhsT=dummy_sb[:, 0:128], rhs=dummy_sb[:],
                                 start=True, stop=True)

            # ---- prologue DMAs: first q-matmul needs only wf's first
            # column block + x chunk 0; everything else streams behind.
            # Critical-path DMAs ride the sync queue; bulk weights + the
            # next s-chunk ride the scalar queue in parallel ----
            ones_sb = pers.tile([128, 128], BF16, tag="ones")
            xs_tiles = []
            xs0 = xs_pool.tile([128, KC, IT], BF16, tag="xs", name="xs0")
            xs_tiles.append(xs0)
            nc.sync.dma_start(wq_sb[:, 0:DG], wf[:, 0:DG])
            nc.scalar.dma_start(wk_sb[:, 0:DG], wf[:, DG:2 * DG])
            for k in range(KC):
                eng = nc.sync if k % 2 == 0 else nc.scalar
                eng.dma_start(xs0[:, k, :], xp[:, k * IT:(k + 1) * IT])
            nc.sync.dma_start(wq_sb[:, DG:], wq[:, DG:])
            nc.scalar.dma_start(wk_sb[:, DG:], wk[:, DG:])
            nc.sync.dma_start(ones_sb[:], wf[:, 2 * DG:])
            for k in range(KC):
                nc.scalar.dma_start(wv_sb[:, k * DG:(k + 1) * DG],
                                    wv[:, k * DG:(k + 1) * DG])
            xs1 = xs_pool.tile([128, KC, IT], BF16, tag="xs", name="xs1")
            xs_tiles.append(xs1)
            nc.sync.dma_start(xs1[:], xp[:, KC * IT:2 * KC * IT])
            nc.scalar.dma_start(wo_sb[:], wo[:])

            # ---- attention pipeline state machine ----
            ctx_q = []        # (emit_fn, group_end_fn | None)
            op_q = []         # pending out-proj emitters (prev query block)
            norms_open = [0]  # cx writes not yet emitted
            op_gate = [True]  # hold out-proj pops for the pair1 stretch

            def tick():
                if op_q and norms_open[0] == 0 and op_gate[0]:
                    op_q.pop(0)()

            def drain_ctx():
                emit, group_end = ctx_q.pop(0)
                emit()
                if group_end is not None:
                    group_end()

            cur = {0: {}, 1: {}}  # cps / pden tiles per pair-group in flight

            def emit_norm(cps, pden, p, ti):
                # ones[128,64]^T @ pden: reduces over the 128 key partitions
                # AND broadcasts den to 64 partitions per head, col-tiled so
                # both heads run concurrently in one PE pass. Then recip +
                # one [128,512] mul normalizes both heads of the pair.
                bc = pj_pool.tile([128, IT], F32, name="dnb", tag="pj")
                for e in range(2):
                    nc.tensor.matmul(bc[e * E:(e + 1) * E, :],
                                     lhsT=ones_sb[:, 0:E], rhs=pden[:, e, :],
                                     start=True, stop=True)
                rc = dn_pool.tile([128, IT], F32, tag="rc")
                nc.vector.reciprocal_approx_fast(rc[:, :], bc[:, :])
                nc.vector.tensor_mul(
                    cx_sb[:, p * S + ti * IT: p * S + (ti + 1) * IT],
                    cps[:, :], rc[:, :])
                norms_open[0] -= 1

            def att_unit(ti, p, jj, njt):
                def go():
                    d = jj - 4 * ti
                    o = max(d, 0) * JT
                    sp = sp_pool.tile([128, 2, IT], F32, tag="sp")
                    for e in range(2):
                        # row-tiled K=64 pair: e=0 rows 0:64 / e=1 rows 64:128
                        # of the PE array run concurrently
                        nc.tensor.matmul(
                            sp[:, e, o:IT],
                            lhsT=kT_sb[e * E:(e + 1) * E, p * S + jj * JT: p * S + jj * JT + JT],
                            rhs=qT_sb[e * E:(e + 1) * E, p * S + ti * IT + o: p * S + (ti + 1) * IT],
                            start=True, stop=True,
                        )
                    pt = pt_pool.tile([128, 2, IT], BF16, tag="pt")
                    nc.scalar.activation(pt[:, :, o:IT], sp[:, :, o:IT], EXP, scale=SCALE)
                    if d >= 0:
                        # keep where query_pos >= key_pos; only the leading
                        # 128-wide band of the region can be masked
                        nc.gpsimd.affine_select(
                            pt[:, :, o:o + JT], pt[:, :, o:o + JT],
                            pattern=[[0, 2], [1, JT]],
                            compare_op=mybir.AluOpType.is_ge,
                            fill=zero_reg,
                            base=0,
                            channel_multiplier=-1,
                        )
                    if jj == 0:
                        cur[p]["pden"] = dn_pool.tile([128, 2, IT], BF16,
                                                      name="pden", tag="pden")
                        nc.vector.tensor_copy(cur[p]["pden"][:], pt[:])
                    else:
                        nc.vector.tensor_add(cur[p]["pden"][:, :, o:IT],
                                             cur[p]["pden"][:, :, o:IT],
                                             pt[:, :, o:IT])

                    def emit_ctx(pt=pt, o=o, jj=jj, njt=njt):
                        if jj == 0:
                            cur[p]["cps"] = cp_pool.tile(
                                [128, IT], F32, name="cps", tag="cps")
                        for e in range(2):
                            # col-tiled M=64 pair: e=0 cols 0:64 / e=1 cols
                            # 64:128 run concurrently, one PSUM bank total
                            nc.tensor.matmul(
                                cur[p]["cps"][e * E:(e + 1) * E, o:IT],
                                lhsT=v_sb[:, jj * VW + (2 * p + e) * E:
                                          jj * VW + (2 * p + e + 1) * E],
                                rhs=pt[:, e, o:IT],
                                start=(jj == 0), stop=(jj == njt - 1),
                            )
                    group_end = None
                    if jj == njt - 1:
                        norms_open[0] += 1
                        # capture pden eagerly: `cur` is clobbered by the next
                        # group's first unit before this group_end drains.
                        # cps is created in the deferred stream, where the
                        # group_end runs before the next group's first ctx.
                        def group_end(p=p, ti=ti, pden=cur[p]["pden"]):
                            emit_norm(cur[p]["cps"], pden, p, ti)
                    ctx_q.append((emit_ctx, group_end))
                    if len(ctx_q) > SK:
                        drain_ctx()
                    tick()
                return go

            def emit_op(ti):
                last = ti == NSC - 1
                def go_all():
                    for it_ in range(4 * ti, 4 * ti + 4):
                        def go(it_=it_):
                            ot = ot_pool.tile([128, 2 * IT], BF16, tag="ott")
                            for dc in range(2):
                                ps = pj_pool.tile([128, IT], F32,
                                                  name="ops", tag="pj")
                                for pr in range(2):
                                    nc.tensor.matmul(
                                        ps[:],
                                        lhsT=cx_sb[:, pr * S + it_ * JT: pr * S + it_ * JT + JT],
                                        rhs=wo_sb[:, pr * D + dc * IT: pr * D + (dc + 1) * IT],
                                        start=(pr == 0), stop=(pr == 1),
                                    )
                                # stage on alternating engines and split the
                                # DMA drain over both HWDGE queues
                                if last and dc == 0:
                                    nc.scalar.copy(ot[:, dc * IT:(dc + 1) * IT], ps[:])
                                else:
                                    nc.vector.tensor_copy(ot[:, dc * IT:(dc + 1) * IT], ps[:])
                                eng = nc.scalar if (last and (it_ + dc) % 2) else nc.sync
                                eng.dma_start(
                                    out[it_ * JT:(it_ + 1) * JT, dc * IT:(dc + 1) * IT],
                                    ot[:, dc * IT:(dc + 1) * IT])
                        op_q.append(go)
                return go_all

            # ---- projection emitters ----
            def emit_fill(n, skip_check=False):
                # dummy matmuls: keep the PE activity monitor busy across
                # DMA-bound holes in phase 0 so the clock never re-gates
                fps = pj_pool.tile([128, IT], F32, name="fill", tag="pj")
                for _ in range(n):
                    nc.tensor.matmul(fps[:], lhsT=dummy_sb[:, 0:128],
                                     rhs=dummy_sb[:], start=True, stop=True,
                                     skip_group_check=skip_check)

            def qk_group(sc, pair, which):
                def go():
                    ps = pj_pool.tile([128, IT], F32, name="pjt", tag="pj")
                    w_sb = wq_sb if which == "q" else wk_sb
                    for k in range(KC):
                        nc.tensor.matmul(
                            ps[:],
                            lhsT=w_sb[:, k * DG + pair * 128: k * DG + (pair + 1) * 128],
                            rhs=xs_tiles[sc][:, k, :],
                            start=(k == 0), stop=(k == KC - 1),
                        )
                    dst = qT_sb if which == "q" else kT_sb
                    # DVE: the scalar engine is saturated by exp; the norm
                    # chain has a group of slack via the 2-deep cps ring
                    nc.vector.tensor_copy(
                        dst[:, pair * S + sc * IT: pair * S + (sc + 1) * IT], ps[:])
                    if sc == 0:
                        emit_fill(2)
                return go

            def v_group(sc, st):
                def go():
                    ps = pj_pool.tile([128, IT], F32, name="pjt", tag="pj")
                    for k in range(KC):
                        nc.tensor.matmul(
                            ps[:, 0:DG],
                            lhsT=xs_tiles[sc][:, k, st * JT:(st + 1) * JT],
                            rhs=wv_sb[:, k * DG:(k + 1) * DG],
                            start=(k == 0), stop=(k == KC - 1),
                        )
                    nc.scalar.copy(
                        v4[:, sc * 4 + st, :, :],
                        ps[:, 0:DG].rearrange("p (h e) -> p h e", e=E))
                    if sc == 0:
                        emit_fill(2)
                return go

            # ---- phase driver ----
            def merge(groups, units, gcyc, ucyc):
                tp, tu = len(groups) * gcyc, len(units) * ucyc
                pc = uc = 0
                while groups or units:
                    if groups and (not units or pc * tu <= uc * tp):
                        groups.pop(0)()
                        pc += gcyc
                    else:
                        units.pop(0)()
                        uc += ucyc


            for ti in range(NSC):
                sc = ti
                if sc + 2 < NSC:
                    xs_n = xs_pool.tile([128, KC, IT], BF16, tag="xs",
                                        name=f"xs{sc + 2}")
                    xs_tiles.append(xs_n)
                    nc.scalar.dma_start(
                        xs_n[:], xp[:, (sc + 2) * KC * IT:(sc + 3) * KC * IT])
                njt = 4 * (ti + 1)
                op_gate[0] = False
                qk_group(sc, 0, "q")()
                qk_group(sc, 0, "k")()
                # pair0 off-diagonal units interleaved with remaining groups
                rest = [qk_group(sc, 1, "q"), qk_group(sc, 1, "k"),
                        v_group(sc, 0), v_group(sc, 1),
                        v_group(sc, 2), v_group(sc, 3)]
                p0_off = [att_unit(ti, 0, jj, njt) for jj in range(4 * ti)]
                merge(rest, p0_off, 3072, 1024)
                op_gate[0] = True   # out-proj fills the group-free stretch
                if ti == NSC - 1:
                    # endgame: weave pair0's diagonals into pair1's stream so
                    # the exp->mask->ctx latency chains of the two groups
                    # pipeline instead of running back-to-back
                    units = [att_unit(ti, 1, jj, njt) for jj in range(njt)]
                    for k, jj in enumerate(range(4 * ti, njt)):
                        units.insert(5 * k, att_unit(ti, 0, jj, njt))
                    for u in units:
                        u()
                else:
                    for jj in range(4 * ti, njt):      # pair0 diagonal
                        att_unit(ti, 0, jj, njt)()
                    for jj in range(njt):              # pair1
                        att_unit(ti, 1, jj, njt)()
                emit_op(ti)()

            while ctx_q:
                drain_ctx()
                tick()
            for _ in range(80):
                if not op_q:
                    break
                tick()
            assert not op_q and norms_open[0] == 0

    nc.compile()
    return nc


def _pack_x(xb):
    # x[b] [2048, 1024] -> [128, 4*8*512]: chunk (sc, k) = xT[k*128:+128, sc*512:+512]
    return np.ascontiguousarray(
        xb.reshape(NSC, IT, KC, 128).transpose(3, 0, 2, 1)
        .reshape(128, NSC * KC * IT).astype(ml_dtypes.bfloat16))


def _pack_w(w):
    # [1024, 256] -> [128, 8*256] chunk-major
    return np.ascontiguousarray(
        w.reshape(KC, 128, DG).transpose(1, 0, 2)
        .reshape(128, KC * DG).astype(ml_dtypes.bfloat16))


def _pack_wo(w):
    # [256, 1024] -> [128, 2*1024] pair-major
    return np.ascontiguousarray(
        w.reshape(2, 128, D).transpose(1, 0, 2)
        .reshape(128, 2 * D).astype(ml_dtypes.bfloat16))


def _in_maps(x, Wq, Wk, Wv, Wo):
    maps = []
    ones = np.ones((128, 128), ml_dtypes.bfloat16)
    for c in range(NC):
        b, g = c // (NC // B), c % (NC // B)
        wqp = _pack_w(Wq[:, g * DG:(g + 1) * DG])
        wkp = _pack_w(Wk[:, g * DG:(g + 1) * DG])
        maps.append({
            "xp": _pack_x(x[b]),
            "wq": wqp,
            "wk": wkp,
            "wv": _pack_w(Wv[:, g * DG:(g + 1) * DG]),
            "wo": _pack_wo(Wo[g * DG:(g + 1) * DG, :]),
            "wf": np.ascontiguousarray(
                np.concatenate([wqp[:, 0:DG], wkp[:, 0:DG], ones], axis=1)),
        })
    return maps


def run(x, Wq, Wk, Wv, Wo, bo, trace=False):
    if "nc" not in _CACHE:
        _CACHE["nc"] = _build()
    nc = _CACHE["nc"]
    res = bass_utils.run_bass_kernel_spmd(
        nc, _in_maps(x, Wq, Wk, Wv, Wo), core_ids=list(range(NC)), trace=trace,
    )
    parts = [np.asarray(res.results[c]["out"], np.float32) for c in range(NC)]
    gpb = NC // B
    full = np.stack([sum(parts[b * gpb + 1: (b + 1) * gpb], parts[b * gpb]) for b in range(B)])
    full = full + np.asarray(bo, np.float32)[None, None, :]
    return full.astype(np.float32), res


def kernel(x, Wq, Wk, Wv, Wo, bo):
    x = np.asarray(x, np.float32)
    full, _ = run(x, np.asarray(Wq, np.float32), np.asarray(Wk, np.float32),
                  np.asarray(Wv, np.float32), np.asarray(Wo, np.float32),
                  np.asarray(bo, np.float32))
    return full
